# revision 38
# baseline (speedup 1.0000x reference)
"""AnomalyAttention (two causal attentions per (b,h)) on 8 TRN2 NeuronCores.

Sharding: B*H = 16 (batch, head) pairs -> 2 pairs per core. Each core runs
4 independent causal attentions (time + channel for each of its 2 pairs).
No cross-core communication.

Per-attention layout ("transposed flash"): keys live on SBUF partitions.
  S^T[k, q]   = kT_tile.T @ qT                (PE bf16, contraction E=64;
                the two attn types use PE row-groups 0-63 / 64-127)
  P^T         = exp(scale * S^T)              (ACT, PSUM -> SBUF bf16)
  diag mask   = affine_select zero-triangle   (GPSIMD, on the diag block)
  outT[d, q] += V_ext[k, d].T @ P^T[k, q]     (PE bf16, PSUM-accumulated)
V_ext carries a ones column so row 64 of outT accumulates the softmax
denominator.

Queries are processed in 512-wide passes (one PSUM bank per tile):
scores for both attn types share one [128,2,512] psum tile (3 bufs) so
one exp instruction covers both and the t=1 score matmul pairs with t=0
on disjoint PE row tiles; PV accumulators are [65,512] (2 tags x 1 buf).
The (g, pass, keytile) iteration is one flat software-pipelined loop
with PV matmuls trailing the scores by DEPTH=4 key tiles so every PE
semaphore wait is stale at decode (a freshly-satisfied wait costs
~120ns of sequencer stall); at pass boundaries no PV is popped so the
next pass's first exp input is ready without an ACT bubble.
Epilogue per (pair, pass): drain both PV psum tiles to SBUF (releases
the banks for the next pass), then per type: denominator row to
partition 0, fast-reciprocal, replicate to partition 32, stream_shuffle
quadrant-broadcast to 0..63, multiply, and a combined bf16 store with
2KB per-partition lines split over 4 DGE queues. Input DMAs are staged
in first-use order so the PE starts after ~0.5 MB instead of 3 MB.
"""

import math
from contextlib import ExitStack

import ml_dtypes
import numpy as np

import concourse.bacc as bacc
import concourse.mybir as mybir
import concourse.tile as tile
from concourse.bass_utils import run_bass_kernel_spmd

B, L, H, E, D = 2, 2048, 8, 64, 64
NCORES = 8
PAIRS = (B * H) // NCORES          # (b,h) pairs per core = 2
NATT = 2 * PAIRS                   # attentions per core = 4
SCALE = 1.0 / math.sqrt(E)
P = 128                            # partitions / key-tile size
NKT = L // P                       # 16 key tiles
Q = 512                            # query pass width (one PSUM bank)
NQP = L // Q                       # 4 query passes
F32 = mybir.dt.float32
BF16 = mybir.dt.bfloat16

_CACHE = {}


def _build_nc():
    nc = bacc.Bacc()
    qt = nc.declare_dram_parameter("qt", [P, PAIRS, L], BF16, isOutput=False)
    first = nc.declare_dram_parameter("first", [P, 3 * Q], BF16, isOutput=False)
    kt = nc.declare_dram_parameter("kt", [P, PAIRS, L], BF16, isOutput=False)
    ve = nc.declare_dram_parameter("ve", [P, NATT, NKT, D + 1], BF16, isOutput=False)
    out = nc.declare_dram_parameter("out", [PAIRS, NQP, D, 2, Q], BF16, isOutput=True)

    with tile.TileContext(nc) as tc:
        with ExitStack() as ctx:
            _body(ctx, tc, qt, kt, ve, first, out)
    nc.finalize()
    return nc


def _body(ctx, tc, qt, kt, ve, first, out):
    nc = tc.nc
    Exp = mybir.ActivationFunctionType.Exp

    persist = ctx.enter_context(tc.tile_pool(name="persist", bufs=1))
    # 8 PSUM banks: scores 2 tags x 3 bufs (one bank each) + pv 2 tags x 1
    s_psum = ctx.enter_context(tc.tile_pool(name="s_psum", bufs=3, space="PSUM"))
    pv_psum = ctx.enter_context(tc.tile_pool(name="pv_psum", bufs=1, space="PSUM"))
    p_pool = ctx.enter_context(tc.tile_pool(name="p_pool", bufs=8))
    o_pool = ctx.enter_context(tc.tile_pool(name="o_pool", bufs=3))
    small = ctx.enter_context(tc.tile_pool(name="small", bufs=3))

    qt_sb = persist.tile([P, PAIRS, L], BF16)
    kt_sb = persist.tile([P, PAIRS, L], BF16)
    ve_sb = persist.tile([P, NATT, NKT, D + 1], BF16)
    first_sb = persist.tile([P, 3 * Q], BF16)

    # staged input loads, first-use order, round-robin over 4 DGE queues.
    # chunks keep >=2KB per-partition lines for DMA efficiency; the first
    # (critical) kt/qt pieces are split by partition halves so two queues
    # work on each
    queues = [nc.default_dma_engine, nc.scalar, nc.gpsimd, nc.sync]
    qi = 0

    def dma_in(dst, src):
        nonlocal qi
        queues[qi % len(queues)].dma_start(out=dst, in_=src)
        qi += 1

    # gate piece: kt g0 keys 0:1024 ++ qt g0 queries 0:512 packed by the
    # host into one tensor -> 3KB per-partition lines, 32 lines per queue
    # (half the line count of separate kt/qt pieces). The first-pass reads
    # come from first_sb; kt_sb/qt_sb skip the duplicated ranges.
    for ph in range(4):
        p0, p1 = 32 * ph, 32 * ph + 32
        dma_in(first_sb[p0:p1, :], first[p0:p1, :])
    dma_in(qt_sb[:, 0, Q:2 * Q], qt[:, 0, Q:2 * Q])
    for a in range(2):
        dma_in(ve_sb[:, a, :, :], ve[:, a, :, :])
    dma_in(kt_sb[:, 0, 2 * Q:L], kt[:, 0, 2 * Q:L])
    dma_in(qt_sb[:, 0, 2 * Q:L], qt[:, 0, 2 * Q:L])
    for a in range(2, 4):
        dma_in(ve_sb[:, a, :, :], ve[:, a, :, :])
    for c in range(2):
        dma_in(kt_sb[:, 1, Q * 2 * c:Q * 2 * (c + 1)], kt[:, 1, Q * 2 * c:Q * 2 * (c + 1)])
        dma_in(qt_sb[:, 1, Q * 2 * c:Q * 2 * (c + 1)], qt[:, 1, Q * 2 * c:Q * 2 * (c + 1)])

    pv_tiles = {}

    def emit_pv(g, qs, k, pTk, w, off):
        last = k == 4 * qs + 3
        for t in range(2):
            nc.tensor.matmul(
                pv_tiles[(g, qs)][t][:, off:Q],
                lhsT=ve_sb[:, 2 * g + t, k, :],
                rhs=pTk[:, t, :w],
                start=(k == 0),
                stop=last,
                skip_group_check=True,
            )
        return last

    def epilogue(g, qs):
        pvs = pv_tiles.pop((g, qs))
        # drain both PSUM tiles to SBUF first: the next pass's first PV
        # matmul WAR-waits on these banks (pv bufs=1), so releasing them
        # after one copy instead of after the whole normalize chain removes
        # the ~1.5us PE gap at every pass boundary
        obs = []
        for t in range(2):
            ob = o_pool.tile([D + 1, Q], F32, tag=f"ob{t}")
            nc.vector.tensor_copy(out=ob, in_=pvs[t])
            obs.append(ob)
        o_n = o_pool.tile([D, 2, Q], BF16, tag="o")
        for t in range(2):
            ob = obs[t]
            # denominator row (partition 64) -> partition 0, reciprocal,
            # replicate to partition 32, then quadrant-broadcast to 0..63
            # (cross-partition-base shifts are fine on plain DVE copies, but
            # NOT on the custom-DVE reciprocal — keep recip at matching base)
            den = small.tile([D, Q], F32, tag="den")
            rec = small.tile([D, Q], F32, tag="rec")
            nc.vector.tensor_copy(out=den[0:1, :], in_=ob[D:D + 1, :])
            nc.vector.reciprocal_approx_fast(out=rec[0:1, :], in_=den[0:1, :])
            nc.vector.tensor_copy(out=rec[32:33, :], in_=rec[0:1, :])
            rec_b = small.tile([D, Q], F32, tag="rec_b")
            nc.vector.stream_shuffle(out=rec_b, in_=rec, mask=[0] * 32)
            nc.vector.tensor_mul(o_n[:, t, :], ob[0:D, :], rec_b)
        # both attn types go out as one [D, 2, Q] tile: 2KB per-partition
        # lines (vs 1KB) halve the per-line DMA overhead; 4-way queue split
        # quarters the per-queue drain
        for p4 in range(4):
            queues[p4].dma_start(
                out=out[g, qs, 16 * p4:16 * p4 + 16, :, :],
                in_=o_n[16 * p4:16 * p4 + 16, :, :],
            )

    # g0 ascending (small first: the DMA gate only covers the first keys),
    # g1 descending: the long (g1,qs3) pass directly follows (g0,qs3), so
    # that epilogue's ~7us DVE chain drains under a 16-tile pass instead of
    # colliding with the 4-tile (g1,qs0) pass (observed ~3.5us ACT bubble)
    qs_order = [list(range(NQP)), list(range(NQP - 1, -1, -1))]
    work = [(g, qs, k) for g in range(PAIRS) for qs in qs_order[g]
            for k in range(4 * qs + 4)]
    # PV matmuls trail the score matmuls by DEPTH key tiles so that every
    # PE instruction's semaphore wait is satisfied well before decode (a
    # freshly-satisfied wait costs ~120ns of sequencer stall per matmul)
    DEPTH = 4
    pend = []
    for g, qs, k in work:
        q0, q1 = qs * Q, qs * Q + Q
        qlo = max(q0, P * k)
        w = q1 - qlo
        off = qlo - q0
        diag = qlo == P * k
        if k == 0:
            pv_tiles[(g, qs)] = [
                pv_psum.tile([D + 1, Q], F32, tag=f"pv{t}", name=f"pv{t}")
                for t in range(2)
            ]
        # both attn types' score matmuls target one [128, 2, Q] psum tile
        # (PE row groups 0-63 / 64-127): the t=1 matmul's psum WAR wait is
        # identical to t=0's, so it is stale at decode and the pair executes
        # concurrently on disjoint PE row tiles
        s2 = s_psum.tile([P, 2, Q], F32, tag="s", name="s")
        for t in range(2):
            bp = 64 * t
            if g == 0 and k < 2 * Q // P:
                lhsT = first_sb[bp:bp + 64, P * k:P * (k + 1)]
            else:
                lhsT = kt_sb[bp:bp + 64, g, P * k:P * (k + 1)]
            if g == 0 and qs == 0:
                rhs = first_sb[bp:bp + 64, 2 * Q + qlo:2 * Q + q1]
            else:
                rhs = qt_sb[bp:bp + 64, g, qlo:q1]
            nc.tensor.matmul(
                s2[:, t, :w],
                lhsT=lhsT,
                rhs=rhs,
                start=True,
                stop=True,
                skip_group_check=True,
            )
        # at a pass boundary (k==0) emit no PV so the new pass's first score
        # pair lands back-to-back with the old pass's last — the ACT engine
        # (the steady-state bottleneck) otherwise idles ~1us waiting for the
        # first exp's input. Afterwards drain old-pass PVs at 2/tile
        # unconditionally: with the default lp>DEPTH rule a short pass never
        # catches up and the previous epilogue (whose ob-copies release the
        # pv PSUM banks) lands a whole pass late, stalling the PV stream
        if k == 0:
            pops = 0
        else:
            pops = min(2, max(0, len(pend) - DEPTH + 1))
        for _ in range(pops):
            pg, pqs, pk, ppT, pw, poff = pend.pop(0)
            if emit_pv(pg, pqs, pk, ppT, pw, poff):
                epilogue(pg, pqs)
        # one exp covers both attn types ([128, 2, w] strided AP) to halve
        # the ACT per-instruction access overhead
        pT = p_pool.tile([P, 2, Q], BF16, tag="p", name="p")
        nc.scalar.activation(pT[:, :, :w], s2[:, :, :w], Exp, scale=SCALE)
        if diag:
            # diagonal block, both attn types at once: zero where q < key
            # (iota = j - part, constant across the t dim)
            nc.gpsimd.affine_select(
                out=pT[:, :, 0:P],
                in_=pT[:, :, 0:P],
                compare_op=mybir.AluOpType.is_ge,
                fill=0.0,
                base=0,
                channel_multiplier=-1,
                pattern=[[0, 2], [1, P]],
            )
        pend.append((g, qs, k, pT, w, off))
    for pg, pqs, pk, ppT, pw, poff in pend:
        if emit_pv(pg, pqs, pk, ppT, pw, poff):
            epilogue(pg, pqs)


def _host_shard(inputs):
    """Build the 8 per-core input maps from full inputs (host-side numpy)."""
    q_t = np.asarray(inputs["queries_time"], dtype=np.float32)
    k_t = np.asarray(inputs["keys_time"], dtype=np.float32)
    v_t = np.asarray(inputs["values_time"], dtype=np.float32)
    q_c = np.asarray(inputs["queries_channel"], dtype=np.float32)
    k_c = np.asarray(inputs["keys_channel"], dtype=np.float32)
    v_c = np.asarray(inputs["values_channel"], dtype=np.float32)

    bf16 = ml_dtypes.bfloat16
    in_maps = []
    for c in range(NCORES):
        qt = np.empty((P, PAIRS, L), np.float32)
        kt = np.empty((P, PAIRS, L), np.float32)
        ve = np.empty((P, NATT, NKT, D + 1), np.float32)
        for g in range(PAIRS):
            p = PAIRS * c + g
            b, h = divmod(p, H)
            qt[:64, g, :] = q_t[b, :, h, :].T
            qt[64:, g, :] = q_c[b, :, h, :].T
            kt[:64, g, :] = k_t[b, :, h, :].T
            kt[64:, g, :] = k_c[b, :, h, :].T
            for t, v_full in enumerate((v_t, v_c)):
                a = 2 * g + t
                # ve[p_row, a, ktile, 0:64] = V[ktile*128 + p_row, :]
                ve[:, a, :, :D] = v_full[b, :, h, :].reshape(NKT, P, D).transpose(1, 0, 2)
                ve[:, a, :, D] = 1.0
        first = np.concatenate([kt[:, 0, 0:1024], qt[:, 0, 0:512]], axis=1)
        in_maps.append({
            "qt": np.ascontiguousarray(qt).astype(bf16),
            "kt": np.ascontiguousarray(kt).astype(bf16),
            "ve": np.ascontiguousarray(ve).astype(bf16),
            "first": np.ascontiguousarray(first).astype(bf16),
        })
    return in_maps


def _run(in_maps, trace=False):
    if "nc" not in _CACHE:
        _CACHE["nc"] = _build_nc()
    return run_bass_kernel_spmd(
        _CACHE["nc"], in_maps, core_ids=list(range(NCORES)), trace=trace
    )


def kernel(**inputs):
    in_maps = _host_shard(inputs)
    res = _run(in_maps, trace=False)
    v_time = np.empty((B, L, H, D), np.float32)
    v_chan = np.empty((B, L, H, D), np.float32)
    for c in range(NCORES):
        o = np.asarray(res.results[c]["out"]).astype(np.float32)  # [PAIRS,NQP,D,2,Q]
        for g in range(PAIRS):
            p = PAIRS * c + g
            b, h = divmod(p, H)
            # o[g, qs, d, t, q] -> [t, l=qs*Q+q, d]
            ot = o[g].transpose(2, 0, 3, 1).reshape(2, L, D)
            v_time[b, :, h, :] = ot[0]
            v_chan[b, :, h, :] = ot[1]
    return v_time, v_chan


# revision 39
# speedup vs baseline: 1.0096x; 1.0096x over previous
"""AnomalyAttention (two causal attentions per (b,h)) on 8 TRN2 NeuronCores.

Sharding: B*H = 16 (batch, head) pairs -> 2 pairs per core. Each core runs
4 independent causal attentions (time + channel for each of its 2 pairs).
No cross-core communication.

Per-attention layout ("transposed flash"): keys live on SBUF partitions.
  S^T[k, q]   = kT_tile.T @ qT                (PE bf16, contraction E=64;
                the two attn types use PE row-groups 0-63 / 64-127)
  P^T         = exp(scale * S^T)              (ACT, PSUM -> SBUF bf16)
  diag mask   = affine_select zero-triangle   (GPSIMD, on the diag block)
  outT[d, q] += V_ext[k, d].T @ P^T[k, q]     (PE bf16, PSUM-accumulated)
V_ext carries a ones column so row 64 of outT accumulates the softmax
denominator.

Queries are processed in 512-wide passes (one PSUM bank per tile):
scores for both attn types share one [128,2,512] psum tile (3 bufs) so
one exp instruction covers both and the t=1 score matmul pairs with t=0
on disjoint PE row tiles; PV accumulators are [65,512] (2 tags x 1 buf).
The (g, pass, keytile) iteration is one flat software-pipelined loop
with PV matmuls trailing the scores by DEPTH=4 key tiles so every PE
semaphore wait is stale at decode (a freshly-satisfied wait costs
~120ns of sequencer stall); at pass boundaries no PV is popped so the
next pass's first exp input is ready without an ACT bubble.
Epilogue per (pair, pass): drain both PV psum tiles to SBUF (releases
the banks for the next pass), then per type: denominator row to
partition 0, fast-reciprocal, replicate to partition 32, stream_shuffle
quadrant-broadcast to 0..63, multiply, and a combined bf16 store with
2KB per-partition lines split over 4 DGE queues. Input DMAs are staged
in first-use order so the PE starts after ~0.5 MB instead of 3 MB.
"""

import math
from contextlib import ExitStack

import ml_dtypes
import numpy as np

import concourse.bacc as bacc
import concourse.mybir as mybir
import concourse.tile as tile
from concourse.bass_utils import run_bass_kernel_spmd

B, L, H, E, D = 2, 2048, 8, 64, 64
NCORES = 8
PAIRS = (B * H) // NCORES          # (b,h) pairs per core = 2
NATT = 2 * PAIRS                   # attentions per core = 4
SCALE = 1.0 / math.sqrt(E)
P = 128                            # partitions / key-tile size
NKT = L // P                       # 16 key tiles
Q = 512                            # query pass width (one PSUM bank)
NQP = L // Q                       # 4 query passes
F32 = mybir.dt.float32
BF16 = mybir.dt.bfloat16

_CACHE = {}


def _build_nc():
    nc = bacc.Bacc()
    qt = nc.declare_dram_parameter("qt", [P, PAIRS, L], BF16, isOutput=False)
    first = nc.declare_dram_parameter("first", [P, 3 * Q], BF16, isOutput=False)
    kt = nc.declare_dram_parameter("kt", [P, PAIRS, L], BF16, isOutput=False)
    ve = nc.declare_dram_parameter("ve", [P, NATT, NKT, D + 1], BF16, isOutput=False)
    out = nc.declare_dram_parameter("out", [PAIRS, NQP, D, 2, Q], BF16, isOutput=True)

    with tile.TileContext(nc) as tc:
        with ExitStack() as ctx:
            _body(ctx, tc, qt, kt, ve, first, out)
    nc.finalize()
    return nc


def _body(ctx, tc, qt, kt, ve, first, out):
    nc = tc.nc
    Exp = mybir.ActivationFunctionType.Exp

    persist = ctx.enter_context(tc.tile_pool(name="persist", bufs=1))
    # 8 PSUM banks: scores 2 tags x 3 bufs (one bank each) + pv 2 tags x 1
    s_psum = ctx.enter_context(tc.tile_pool(name="s_psum", bufs=3, space="PSUM"))
    pv_psum = ctx.enter_context(tc.tile_pool(name="pv_psum", bufs=1, space="PSUM"))
    p_pool = ctx.enter_context(tc.tile_pool(name="p_pool", bufs=8))
    o_pool = ctx.enter_context(tc.tile_pool(name="o_pool", bufs=3))
    small = ctx.enter_context(tc.tile_pool(name="small", bufs=3))

    qt_sb = persist.tile([P, PAIRS, L], BF16)
    kt_sb = persist.tile([P, PAIRS, L], BF16)
    ve_sb = persist.tile([P, NATT, NKT, D + 1], BF16)
    first_sb = persist.tile([P, 3 * Q], BF16)

    # staged input loads, first-use order, round-robin over 4 DGE queues.
    # chunks keep >=2KB per-partition lines for DMA efficiency; the first
    # (critical) kt/qt pieces are split by partition halves so two queues
    # work on each
    queues = [nc.default_dma_engine, nc.scalar, nc.gpsimd, nc.sync]
    qi = 0

    def dma_in(dst, src):
        nonlocal qi
        queues[qi % len(queues)].dma_start(out=dst, in_=src)
        qi += 1

    # gate piece: kt g0 keys 0:1024 ++ qt g0 queries 0:512 packed by the
    # host into one tensor -> 3KB per-partition lines, 32 lines per queue
    # (half the line count of separate kt/qt pieces). The first-pass reads
    # come from first_sb; kt_sb/qt_sb skip the duplicated ranges.
    for ph in range(4):
        p0, p1 = 32 * ph, 32 * ph + 32
        dma_in(first_sb[p0:p1, :], first[p0:p1, :])
    dma_in(qt_sb[:, 0, Q:2 * Q], qt[:, 0, Q:2 * Q])
    for a in range(2):
        dma_in(ve_sb[:, a, :, :], ve[:, a, :, :])
    dma_in(kt_sb[:, 0, 2 * Q:L], kt[:, 0, 2 * Q:L])
    dma_in(qt_sb[:, 0, 2 * Q:L], qt[:, 0, 2 * Q:L])
    for a in range(2, 4):
        dma_in(ve_sb[:, a, :, :], ve[:, a, :, :])
    for c in range(2):
        dma_in(kt_sb[:, 1, Q * 2 * c:Q * 2 * (c + 1)], kt[:, 1, Q * 2 * c:Q * 2 * (c + 1)])
        dma_in(qt_sb[:, 1, Q * 2 * c:Q * 2 * (c + 1)], qt[:, 1, Q * 2 * c:Q * 2 * (c + 1)])

    pv_tiles = {}

    def emit_pv(g, qs, k, pTk, w, off):
        last = k == 4 * qs + 3
        for t in range(2):
            nc.tensor.matmul(
                pv_tiles[(g, qs)][t][:, off:Q],
                lhsT=ve_sb[:, 2 * g + t, k, :],
                rhs=pTk[:, t, :w],
                start=(k == 0),
                stop=last,
                skip_group_check=True,
            )
        return last

    def epilogue(g, qs):
        pvs = pv_tiles.pop((g, qs))
        # drain both PSUM tiles to SBUF first: the next pass's first PV
        # matmul WAR-waits on these banks (pv bufs=1), so releasing them
        # after one copy instead of after the whole normalize chain removes
        # the ~1.5us PE gap at every pass boundary
        obs = []
        for t in range(2):
            ob = o_pool.tile([D + 1, Q], F32, tag=f"ob{t}")
            nc.vector.tensor_copy(out=ob, in_=pvs[t])
            obs.append(ob)
        o_n = o_pool.tile([D, 2, Q], BF16, tag="o")
        for t in range(2):
            ob = obs[t]
            # denominator row (partition 64) -> partition 0, reciprocal,
            # replicate to partition 32, then quadrant-broadcast to 0..63
            # (cross-partition-base shifts are fine on plain DVE copies, but
            # NOT on the custom-DVE reciprocal — keep recip at matching base)
            den = small.tile([D, Q], F32, tag="den")
            rec = small.tile([D, Q], F32, tag="rec")
            nc.vector.tensor_copy(out=den[0:1, :], in_=ob[D:D + 1, :])
            nc.vector.reciprocal_approx_fast(out=rec[0:1, :], in_=den[0:1, :])
            nc.vector.tensor_copy(out=rec[32:33, :], in_=rec[0:1, :])
            rec_b = small.tile([D, Q], F32, tag="rec_b")
            nc.vector.stream_shuffle(out=rec_b, in_=rec, mask=[0] * 32)
            nc.vector.tensor_mul(o_n[:, t, :], ob[0:D, :], rec_b)
        # both attn types go out as one [D, 2, Q] tile: 2KB per-partition
        # lines (vs 1KB) halve the per-line DMA overhead; 4-way queue split
        # quarters the per-queue drain
        for p4 in range(4):
            queues[p4].dma_start(
                out=out[g, qs, 16 * p4:16 * p4 + 16, :, :],
                in_=o_n[16 * p4:16 * p4 + 16, :, :],
            )

    work = [(g, qs, k) for g in range(PAIRS) for qs in range(NQP)
            for k in range(4 * qs + 4)]
    # PV matmuls trail the score matmuls by DEPTH key tiles so that every
    # PE instruction's semaphore wait is satisfied well before decode (a
    # freshly-satisfied wait costs ~120ns of sequencer stall per matmul)
    DEPTH = 4
    pend = []
    for g, qs, k in work:
        q0, q1 = qs * Q, qs * Q + Q
        qlo = max(q0, P * k)
        w = q1 - qlo
        off = qlo - q0
        diag = qlo == P * k
        if k == 0:
            pv_tiles[(g, qs)] = [
                pv_psum.tile([D + 1, Q], F32, tag=f"pv{t}", name=f"pv{t}")
                for t in range(2)
            ]
        # both attn types' score matmuls target one [128, 2, Q] psum tile
        # (PE row groups 0-63 / 64-127): the t=1 matmul's psum WAR wait is
        # identical to t=0's, so it is stale at decode and the pair executes
        # concurrently on disjoint PE row tiles
        s2 = s_psum.tile([P, 2, Q], F32, tag="s", name="s")
        for t in range(2):
            bp = 64 * t
            if g == 0 and k < 2 * Q // P:
                lhsT = first_sb[bp:bp + 64, P * k:P * (k + 1)]
            else:
                lhsT = kt_sb[bp:bp + 64, g, P * k:P * (k + 1)]
            if g == 0 and qs == 0:
                rhs = first_sb[bp:bp + 64, 2 * Q + qlo:2 * Q + q1]
            else:
                rhs = qt_sb[bp:bp + 64, g, qlo:q1]
            nc.tensor.matmul(
                s2[:, t, :w],
                lhsT=lhsT,
                rhs=rhs,
                start=True,
                stop=True,
                skip_group_check=True,
            )
        # at a pass boundary (k==0) emit no PV so the new pass's first score
        # pair lands back-to-back with the old pass's last — the ACT engine
        # (the steady-state bottleneck) otherwise idles ~1us waiting for the
        # first exp's input. Afterwards drain old-pass PVs at 2/tile
        # unconditionally: with the default lp>DEPTH rule a short pass never
        # catches up and the previous epilogue (whose ob-copies release the
        # pv PSUM banks) lands a whole pass late, stalling the PV stream
        if k == 0:
            pops = 0
        else:
            pops = min(2, max(0, len(pend) - DEPTH + 1))
        for _ in range(pops):
            pg, pqs, pk, ppT, pw, poff = pend.pop(0)
            if emit_pv(pg, pqs, pk, ppT, pw, poff):
                epilogue(pg, pqs)
        # one exp covers both attn types ([128, 2, w] strided AP) to halve
        # the ACT per-instruction access overhead
        pT = p_pool.tile([P, 2, Q], BF16, tag="p", name="p")
        nc.scalar.activation(pT[:, :, :w], s2[:, :, :w], Exp, scale=SCALE)
        if diag:
            # diagonal block, both attn types at once: zero where q < key
            # (iota = j - part, constant across the t dim)
            nc.gpsimd.affine_select(
                out=pT[:, :, 0:P],
                in_=pT[:, :, 0:P],
                compare_op=mybir.AluOpType.is_ge,
                fill=0.0,
                base=0,
                channel_multiplier=-1,
                pattern=[[0, 2], [1, P]],
            )
        pend.append((g, qs, k, pT, w, off))
    for pg, pqs, pk, ppT, pw, poff in pend:
        if emit_pv(pg, pqs, pk, ppT, pw, poff):
            epilogue(pg, pqs)


def _host_shard(inputs):
    """Build the 8 per-core input maps from full inputs (host-side numpy)."""
    q_t = np.asarray(inputs["queries_time"], dtype=np.float32)
    k_t = np.asarray(inputs["keys_time"], dtype=np.float32)
    v_t = np.asarray(inputs["values_time"], dtype=np.float32)
    q_c = np.asarray(inputs["queries_channel"], dtype=np.float32)
    k_c = np.asarray(inputs["keys_channel"], dtype=np.float32)
    v_c = np.asarray(inputs["values_channel"], dtype=np.float32)

    bf16 = ml_dtypes.bfloat16
    in_maps = []
    for c in range(NCORES):
        qt = np.empty((P, PAIRS, L), np.float32)
        kt = np.empty((P, PAIRS, L), np.float32)
        ve = np.empty((P, NATT, NKT, D + 1), np.float32)
        for g in range(PAIRS):
            p = PAIRS * c + g
            b, h = divmod(p, H)
            qt[:64, g, :] = q_t[b, :, h, :].T
            qt[64:, g, :] = q_c[b, :, h, :].T
            kt[:64, g, :] = k_t[b, :, h, :].T
            kt[64:, g, :] = k_c[b, :, h, :].T
            for t, v_full in enumerate((v_t, v_c)):
                a = 2 * g + t
                # ve[p_row, a, ktile, 0:64] = V[ktile*128 + p_row, :]
                ve[:, a, :, :D] = v_full[b, :, h, :].reshape(NKT, P, D).transpose(1, 0, 2)
                ve[:, a, :, D] = 1.0
        first = np.concatenate([kt[:, 0, 0:1024], qt[:, 0, 0:512]], axis=1)
        in_maps.append({
            "qt": np.ascontiguousarray(qt).astype(bf16),
            "kt": np.ascontiguousarray(kt).astype(bf16),
            "ve": np.ascontiguousarray(ve).astype(bf16),
            "first": np.ascontiguousarray(first).astype(bf16),
        })
    return in_maps


def _run(in_maps, trace=False):
    if "nc" not in _CACHE:
        _CACHE["nc"] = _build_nc()
    return run_bass_kernel_spmd(
        _CACHE["nc"], in_maps, core_ids=list(range(NCORES)), trace=trace
    )


def kernel(**inputs):
    in_maps = _host_shard(inputs)
    res = _run(in_maps, trace=False)
    v_time = np.empty((B, L, H, D), np.float32)
    v_chan = np.empty((B, L, H, D), np.float32)
    for c in range(NCORES):
        o = np.asarray(res.results[c]["out"]).astype(np.float32)  # [PAIRS,NQP,D,2,Q]
        for g in range(PAIRS):
            p = PAIRS * c + g
            b, h = divmod(p, H)
            # o[g, qs, d, t, q] -> [t, l=qs*Q+q, d]
            ot = o[g].transpose(2, 0, 3, 1).reshape(2, L, D)
            v_time[b, :, h, :] = ot[0]
            v_chan[b, :, h, :] = ot[1]
    return v_time, v_chan
